# revision 19
# baseline (speedup 1.0000x reference)
"""Trainium2 Bass kernel for nn_Attention_63866163692087 (vq_codebook).

Math (verified against the reference):
  logits[b,n,t] = a * sum_c Qg[bt,c] * K[n,c] + beta[bt]
    where Qg = (hs^T @ Wq + bq) * rep(a*(hs^T @ Wp + bp)),  a = 1/sqrt(d*H),
          K = cb @ Wk + bk  (Wk folded into the Q side: Qk = Qg @ Wk^T,
          logits = Qk @ cb^T + beta,  beta = Qg @ bk)
  idx = argmax_n logits          (softmax is monotonic)
  z_q[b,:,t] = cb[idx] @ Wv + bv (straight-through output is a pure gather)

Sharding: 8 cores, each takes 1024 rows of the flattened (b, t) axis
(core i -> b = i//2, t-half = i%2). No collectives.

Per-core layouts: everything is computed in a "transposed" orientation so
argmax runs along the DVE free axis and z_q comes out channel-major:
  hsT  [C, TL]     (native layout of hidden_states[b])
  QgT/QkT [C, TL]
  logits tiles [t(128), N] -> DMA to logits_loc [TL, N] (host transposes)
  zqT  [C, TL]     (direct concat on host)
"""

import os
import sys
import types

for _p in ("/opt/trn_rl_repo", "/root/.axon_site/_ro/trn_rl_repo"):
    if os.path.isdir(_p) and _p not in sys.path:
        sys.path.insert(0, _p)

import numpy as np
import ml_dtypes

import concourse.bass as bass
import concourse.mybir as mybir
from concourse.tile import TileContext
from concourse.vector_clock import ScopedClock
from concourse.masks import make_identity
from concourse.bass_utils import run_bass_kernel_spmd

B, C, T, N, H = 4, 512, 2048, 4096, 4
D = C // H                     # 128, head dim == one partition tile
P = 128
NCORES = 8
TL = (B * T) // NCORES         # 1024 local (b,t) rows per core
ALPHA = 1.0 / float(np.sqrt(D) * np.sqrt(H))

F32 = mybir.dt.float32
BF16 = mybir.dt.bfloat16
F32R = mybir.dt.float32r
U32 = mybir.dt.uint32
COPY = mybir.ActivationFunctionType.Copy
IDENT = mybir.ActivationFunctionType.Identity

# main-matmul dtype: "f32r" (1 cyc/row) or "f32" (4 cyc/row, exact)
MM_DT = os.environ.get("BASS_VQ_MMDT", "f32")


def _patch_tail_drain():
    """Spread the kernel-tail drain's frontier waits over single-wait SP nops
    (walrus rejects >4 sync waits on one instruction)."""
    if getattr(TileContext, "_vq_drain_patched", False):
        return

    def _patched_dab(self, tick_clock, wait_clock):
        probe = self.nc.sync.nop()
        wait_clock.add_sem_waits(
            probe.ins, ScopedClock({None: tick_clock.global_clock})
        )
        si = probe.ins.sync_info
        if si is not None and si.on_wait is not None and len(si.on_wait) > 1:
            waits = list(si.on_wait)
            probe.ins.sync_info = mybir.SyncInfo(
                on_wait=waits[:1], on_update=list(si.on_update or [])
            )
            for w in waits[1:]:
                extra = self.nc.sync.nop()
                extra.ins.sync_info = mybir.SyncInfo(on_wait=[w], on_update=[])
        self.nc.sync.drain()
        self.nc.all_engine_barrier()
        assert self.sems is not None
        popped = self.nc._tile_sem_poison_stack.pop()
        assert popped is self._sem_poison
        self.nc.clear_and_free_semaphores(list(self.sems.allocated().values()))
        self.nc.all_engine_barrier()

    TileContext._drain_and_barrier = _patched_dab
    TileContext._vq_drain_patched = True


def _legalize_waits(nc, max_waits=1):
    """Walrus accepts only a limited number of sync waits per instruction
    (fused fp32 matmuls appear to accept just one). Move excess waits onto
    freshly inserted same-engine NOPs immediately before the instruction —
    engines execute in order, so semantics are preserved."""
    k = 0
    for f in nc.m.functions:
        for blk in f.blocks:
            il = blk.instructions
            i = 0
            while i < len(il):
                inst = il[i]
                si = getattr(inst, "sync_info", None)
                eng = getattr(inst, "engine", None)
                if (
                    si is not None
                    and si.on_wait is not None
                    and len(si.on_wait) > max_waits
                    and eng is not None
                    and eng != mybir.EngineType.Unassigned
                ):
                    waits = list(si.on_wait)
                    keep, excess = waits[:max_waits], waits[max_waits:]
                    inst.sync_info = mybir.SyncInfo(
                        on_wait=keep, on_update=list(si.on_update or [])
                    )
                    for w in excess:
                        nop = mybir.InstNoOp(
                            name=f"I-waitfix-{k}", ins=[], outs=[]
                        )
                        k += 1
                        nop.engine = eng
                        nop.sync_info = mybir.SyncInfo(on_wait=[w], on_update=[])
                        il.insert(i, nop)
                        i += 1
                i += 1
    return k


def _build(use_qp_bias, use_bk, use_bv, mm_dt=None):
    """Per-core Bass graph (SPMD across 8 cores).

    mm_dt: 'f32' — exact fp32 matmuls (4 cyc/row) on the logits chain
           'bf3' — bf16 hi/lo 3-product split (3x 1 cyc/row, near-exact)
    """
    if mm_dt is None:
        mm_dt = MM_DT
    _patch_tail_drain()
    nc = bass.Bass()
    E = 5 if use_qp_bias else 4
    CE = E * P
    bf3 = mm_dt == "bf3"
    MDT = BF16 if bf3 else F32
    # (lhs split, rhs split) product passes: x = h + l, drop l*l
    PASSES = [(0, 0), (0, 1), (1, 0)] if bf3 else [(0, 0)]
    NSP = 2 if bf3 else 1

    def dparam(name, shape, dt):
        return nc.declare_dram_parameter(name, shape, dt, isOutput=False)

    if bf3:
        hst_d = [dparam("hsth", [CE, TL], BF16), dparam("hstl", [CE, TL], BF16)]
        cbt_d = [dparam("cbth", [C, N], BF16), dparam("cbtl", [C, N], BF16)]
        wq_d = [dparam("wqh", [CE, C], BF16), dparam("wql", [CE, C], BF16)]
        wp_d = [dparam("wph", [CE, H], BF16), dparam("wpl", [CE, H], BF16)]
        wkt_d = [dparam("wkth", [C, C], BF16), dparam("wktl", [C, C], BF16)]
    else:
        hst_d = [dparam("hst", [CE, TL], F32)]
        cbt_d = [dparam("cbt", [C, N], F32)]
        wq_d = [dparam("wq", [CE, C], F32)]
        wp_d = [dparam("wp", [CE, H], F32)]
        wkt_d = [dparam("wkt", [C, C], F32)]
    cb_d = dparam("cb", [N, C], F32)
    wv_d = dparam("wv", [C, C], BF16)
    if use_bk:
        bk_d = dparam("bk", [C, 1], F32)
    if use_bv:
        bv_d = dparam("bv", [C, 1], F32)

    logits_d = nc.declare_dram_parameter("logits", [TL, N], F32, isOutput=True)
    idx_d = nc.declare_dram_parameter("idx", [TL, 1], U32, isOutput=True)
    zqt_d = nc.declare_dram_parameter("zqt", [C, TL], F32, isOutput=True)

    with TileContext(nc) as tc:
        with (
            tc.tile_pool(name="const", bufs=1) as const,
            tc.tile_pool(name="work", bufs=1) as work,
            tc.tile_pool(name="gates", bufs=2) as gpool,
            tc.tile_pool(name="lrow", bufs=2) as lpool,
            tc.tile_pool(name="small", bufs=2) as spool,
            tc.tile_pool(name="zq", bufs=2) as zpool,
            tc.tile_pool(name="psmm", bufs=4, space="PSUM") as psmm,
            tc.tile_pool(name="pstp", bufs=2, space="PSUM") as pstp,
            tc.tile_pool(name="pssm", bufs=2, space="PSUM") as pssm,
        ):
            # ---------------- loads ----------------
            hst, wq, wp4, wkt, cbt = {}, {}, {}, {}, {}
            for sp in range(NSP):
                hst[sp] = [const.tile([P, TL], MDT, name=f"hst{sp}_{e}") for e in range(E)]
                wq[sp] = [const.tile([P, C], MDT, name=f"wq{sp}_{e}") for e in range(E)]
                wp4[sp] = [const.tile([P, H], MDT, name=f"wp4{sp}_{e}") for e in range(E)]
                wkt[sp] = [const.tile([P, C], MDT, name=f"wkt{sp}_{c}") for c in range(4)]
                cbt[sp] = [const.tile([P, N], MDT, name=f"cbt{sp}_{e}") for e in range(4)]
            # interleave h/l loads: the x3 passes need both splits, so issue
            # them adjacently (stage-A order first, then wkt, then cbt)
            for e in range(E):
                for sp in range(NSP):
                    nc.sync.dma_start(hst[sp][e][:], hst_d[sp][e * P:(e + 1) * P, :])
                    nc.sync.dma_start(wp4[sp][e][:], wp_d[sp][e * P:(e + 1) * P, :])
                    nc.sync.dma_start(wq[sp][e][:], wq_d[sp][e * P:(e + 1) * P, :])
            for c4 in range(4):
                for sp in range(NSP):
                    nc.sync.dma_start(wkt[sp][c4][:], wkt_d[sp][c4 * P:(c4 + 1) * P, :])
            for c4 in range(4):
                for sp in range(NSP):
                    nc.sync.dma_start(
                        cbt[sp][c4][:], cbt_d[sp][c4 * P:(c4 + 1) * P, :]
                    )
            wv = [const.tile([P, C], BF16, name=f"wv{c}") for c in range(4)]
            for c4 in range(4):
                nc.sync.dma_start(wv[c4][:], wv_d[c4 * P:(c4 + 1) * P, :])
            ident = const.tile([P, P], F32, name="ident")
            make_identity(nc, ident[:])
            identb = const.tile([P, P], BF16, name="identb")
            nc.vector.tensor_copy(identb[:], ident[:])
            if use_bk:
                bks = const.tile([P, 4], F32, name="bks")
                nc.sync.dma_start(bks[:], bk_d.rearrange("(c p) o -> p (c o)", p=P))
                if bf3:
                    # beta matmul operands must match qgt dtype (bf16);
                    # bk's low bf16 bits are dropped (bk==0 in practice)
                    bks_m = const.tile([P, 4], BF16, name="bksm")
                    nc.vector.tensor_copy(bks_m[:], bks[:])
                else:
                    bks_m = bks
            if use_bv:
                bvs = const.tile([P, 4], F32, name="bvs")
                nc.sync.dma_start(bvs[:], bv_d.rearrange("(c p) o -> p (c o)", p=P))

            # device-split QgT / QkT (h/l in bf16 when bf3, else plain f32)
            qgt = {sp: [work.tile([P, TL], MDT, name=f"qgt{sp}_{c}") for c in range(4)]
                   for sp in range(NSP)}
            qkt = {sp: [work.tile([P, TL], MDT, name=f"qkt{sp}_{c}") for c in range(4)]
                   for sp in range(NSP)}
            cbgt = [work.tile([P, TL], BF16, name=f"cbgt{c}") for c in range(4)]
            g4a = work.tile([P, 8 * H], F32, name="g4a")
            if use_bk:
                beta = work.tile([P, 8], F32, name="beta")

            # ------- gates: G[t, h] per t-chunk, then broadcast-transpose ----
            for i in range(8):
                g4p = pssm.tile([P, H], F32, tag="pss", name="g4p")
                nmm = len(PASSES) * E
                k = 0
                for (a, b) in PASSES:
                    for e in range(E):
                        nc.tensor.matmul(
                            g4p[:], lhsT=hst[b][e][:, i * P:(i + 1) * P],
                            rhs=wp4[a][e][:],
                            start=(k == 0), stop=(k == nmm - 1),
                        )
                        k += 1
                nc.scalar.activation(g4a[:, i * H:(i + 1) * H], g4p[:], COPY)

            # ---------------- stage A: QgT / beta / QkT ----------------
            for s in range(2):
                tsl = slice(s * 512, (s + 1) * 512)
                gsb = []
                for h in range(4):
                    grp = psmm.tile([P, 512], F32, tag="ps", name="grp")
                    for j in range(4):
                        i = s * 4 + j
                        nc.tensor.transpose(
                            grp[:, j * P:(j + 1) * P],
                            g4a[:, i * H + h:i * H + h + 1].to_broadcast([P, P]),
                            ident[:],
                        )
                    g = gpool.tile([P, 512], F32, tag="g", name="g")
                    nc.scalar.activation(g[:], grp[:], COPY)
                    gsb.append(g)
                for c4 in range(4):
                    qp = psmm.tile([P, 512], F32, tag="ps", name="qp")
                    nmm = len(PASSES) * E
                    k = 0
                    for (a, b) in PASSES:
                        for e in range(E):
                            nc.tensor.matmul(
                                qp[:], lhsT=wq[a][e][:, c4 * P:(c4 + 1) * P],
                                rhs=hst[b][e][:, tsl],
                                start=(k == 0), stop=(k == nmm - 1),
                            )
                            k += 1
                    if bf3:
                        qg32 = gpool.tile([P, 512], F32, tag="qg32", name="qg32")
                        nc.vector.tensor_mul(qg32[:], qp[:], gsb[c4][:])
                        nc.scalar.activation(qgt[0][c4][:, tsl], qg32[:], COPY)
                        nc.vector.tensor_sub(
                            qgt[1][c4][:, tsl], qg32[:], qgt[0][c4][:, tsl]
                        )
                    else:
                        nc.vector.tensor_mul(qgt[0][c4][:, tsl], qp[:], gsb[c4][:])
                if use_bk:
                    for i in range(s * 4, s * 4 + 4):
                        bp = pssm.tile([P, H], F32, tag="pss", name="bp")
                        nmm = 4 * NSP
                        k = 0
                        for sp in range(NSP):
                            for c4 in range(4):
                                nc.tensor.matmul(
                                    bp[:, 0:1],
                                    lhsT=qgt[sp][c4][:, i * P:(i + 1) * P],
                                    rhs=bks_m[:, c4:c4 + 1],
                                    start=(k == 0), stop=(k == nmm - 1),
                                )
                                k += 1
                        nc.scalar.activation(beta[:, i:i + 1], bp[:, 0:1], COPY)
                for e4 in range(4):
                    kp = psmm.tile([P, 512], F32, tag="ps", name="kp")
                    nmm = len(PASSES) * 4
                    k = 0
                    for (a, b) in PASSES:
                        for c4 in range(4):
                            nc.tensor.matmul(
                                kp[:], lhsT=wkt[a][c4][:, e4 * P:(e4 + 1) * P],
                                rhs=qgt[b][c4][:, tsl],
                                start=(k == 0), stop=(k == nmm - 1),
                            )
                            k += 1
                    nc.scalar.activation(qkt[0][e4][:, tsl], kp[:], COPY)
                    if bf3:
                        nc.vector.tensor_sub(
                            qkt[1][e4][:, tsl], kp[:], qkt[0][e4][:, tsl]
                        )

            # ---------------- stage C helper: z_q for one t-half -------------
            def stage_c(s):
                tsl_c = slice(s * 512, (s + 1) * 512)
                for c4 in range(4):
                    zp = psmm.tile([P, 512], F32, tag="ps", name="zp")
                    for e4 in range(4):
                        nc.tensor.matmul(
                            zp[:], lhsT=wv[e4][:, c4 * P:(c4 + 1) * P],
                            rhs=cbgt[e4][:, tsl_c],
                            start=(e4 == 0), stop=(e4 == 3),
                        )
                    z = zpool.tile([P, 512], F32, tag="z", name="z")
                    if use_bv:
                        nc.scalar.activation(z[:], zp[:], IDENT, bias=bvs[:, c4:c4 + 1])
                    else:
                        nc.scalar.activation(z[:], zp[:], COPY)
                    nc.sync.dma_start(zqt_d[c4 * P:(c4 + 1) * P, tsl_c], z[:])

            # ---------------- stage B: logits, argmax, gather ----------------
            cbgs = []
            for i in range(8):
                tsl = slice(i * P, (i + 1) * P)
                lsb = lpool.tile([P, N], F32, tag="l", name="l")
                for n8 in range(8):
                    nsl = slice(n8 * 512, (n8 + 1) * 512)
                    lp = psmm.tile([P, 512], F32, tag="ps", name="lp")
                    nmm = len(PASSES) * 4
                    k = 0
                    for (a, b) in PASSES:
                        for e4 in range(4):
                            nc.tensor.matmul(
                                lp[:], lhsT=qkt[a][e4][:, tsl],
                                rhs=cbt[b][e4][:, nsl],
                                start=(k == 0), stop=(k == nmm - 1),
                            )
                            k += 1
                    if use_bk:
                        nc.scalar.activation(
                            lsb[:, nsl], lp[:], IDENT, bias=beta[:, i:i + 1]
                        )
                    else:
                        nc.scalar.activation(lsb[:, nsl], lp[:], COPY)
                nc.sync.dma_start(logits_d[tsl, :], lsb[:])
                m8 = spool.tile([P, 8], F32, tag="m8", name="m8")
                i8 = spool.tile([P, 8], U32, tag="i8", name="i8")
                nc.vector.max(out=m8[:], in_=lsb[:])
                nc.vector.max_index(out=i8[:], in_max=m8[:], in_values=lsb[:])
                nc.sync.dma_start(idx_d[tsl, :], i8[:, 0:1])
                cbg = spool.tile([P, C], F32, tag="cbg", name="cbg")
                nc.gpsimd.indirect_dma_start(
                    out=cbg[:], out_offset=None, in_=cb_d[:, :],
                    in_offset=bass.IndirectOffsetOnAxis(ap=i8[:, 0:1], axis=0),
                )
                cbgb = work.tile([P, C], BF16, name=f"cbgb{i}")
                nc.vector.tensor_copy(cbgb[:], cbg[:])
                cbgs.append(cbgb)

            # ---- deferred z_q tail: keep gather->transpose->zmm off the
            # per-chunk PE critical path (PE is in-order; a transpose waiting
            # on chunk i's gather would block chunk i+1's matmuls)
            for i in range(8):
                tsl = slice(i * P, (i + 1) * P)
                for e4 in range(4):
                    tp = pstp.tile([P, P], BF16, tag="pst", name="tp")
                    nc.tensor.transpose(tp[:], cbgs[i][:, e4 * P:(e4 + 1) * P], identb[:])
                    nc.scalar.activation(cbgt[e4][:, tsl], tp[:], COPY)
                if i == 3:
                    stage_c(0)
                elif i == 7:
                    stage_c(1)



    _legalize_waits(nc)
    return nc


def _prep_inputs(inputs):
    """Host-side prep shared by all cores + per-core hs slices."""
    hs = np.ascontiguousarray(np.asarray(inputs["hidden_states"], dtype=np.float32))
    cb = np.ascontiguousarray(
        np.asarray(inputs["codebook_hidden_states"], dtype=np.float32)
    )
    Wq = np.asarray(inputs["Wq"], dtype=np.float32)
    bq = np.asarray(inputs["bq"], dtype=np.float32)
    Wk = np.asarray(inputs["Wk"], dtype=np.float32)
    bk = np.asarray(inputs["bk"], dtype=np.float32)
    Wv = np.asarray(inputs["Wv"], dtype=np.float32)
    bv = np.asarray(inputs["bv"], dtype=np.float32)
    Wp = np.asarray(inputs["Wp"], dtype=np.float32)
    bp = np.asarray(inputs["bp"], dtype=np.float32)

    use_qp_bias = bool(np.any(bq != 0) or np.any(bp != 0))
    use_bk = bool(np.any(bk != 0))
    use_bv = bool(np.any(bv != 0))

    E = 5 if use_qp_bias else 4
    CE = E * P

    wq_eff = np.zeros((CE, C), np.float32)
    wq_eff[:C] = Wq
    wp_eff = np.zeros((CE, H), np.float32)
    wp_eff[:C] = ALPHA * Wp
    if use_qp_bias:
        wq_eff[C] = bq
        wp_eff[C] = ALPHA * bp

    cbt = np.ascontiguousarray(cb.T)
    wkt = np.ascontiguousarray(Wk.T)
    wv_bf = Wv.astype(ml_dtypes.bfloat16)
    bf3 = MM_DT == "bf3"

    def split_bf(x):
        h = x.astype(ml_dtypes.bfloat16)
        l = (x - h.astype(np.float32)).astype(ml_dtypes.bfloat16)
        return h, l

    common = {"cb": cb, "wv": wv_bf}
    if bf3:
        for nm, arr in (("cbt", cbt), ("wq", wq_eff), ("wp", wp_eff),
                        ("wkt", wkt)):
            h, l = split_bf(arr)
            common[nm + "h"] = h
            common[nm + "l"] = l
    else:
        common.update({"cbt": cbt, "wq": wq_eff, "wp": wp_eff, "wkt": wkt})
    if use_bk:
        common["bk"] = bk.reshape(C, 1)
    if use_bv:
        common["bv"] = bv.reshape(C, 1)

    in_maps = []
    for core in range(NCORES):
        b, half = core // 2, core % 2
        hstc = np.zeros((CE, TL), np.float32)
        hstc[:C] = hs[b][:, half * TL:(half + 1) * TL]
        if use_qp_bias:
            hstc[C] = 1.0
        if bf3:
            h, l = split_bf(hstc)
            in_maps.append({"hsth": h, "hstl": l, **common})
        else:
            in_maps.append({"hst": hstc, **common})
    return in_maps, (use_qp_bias, use_bk, use_bv)


def _assemble(results):
    logits = np.empty((B, N, T), np.float32)
    idx = np.empty((B, 1, T), np.int32)
    z_q = np.empty((B, C, T), np.float32)
    for core in range(NCORES):
        b, half = core // 2, core % 2
        tsl = slice(half * TL, (half + 1) * TL)
        r = results[core]
        logits[b][:, tsl] = r["logits"].T
        idx[b, 0, tsl] = r["idx"][:, 0].astype(np.int64).astype(np.int32)
        z_q[b][:, tsl] = r["zqt"]
    return logits, idx, z_q


def _inject_ntff_hook():
    import antenv
    if "antenv.axon_hooks" in sys.modules:
        return
    m = types.ModuleType("antenv.axon_hooks")
    m._hook = None
    def _set(h):
        m._hook = h
    def _get():
        return m._hook
    m.set_axon_ntff_profile_hook = _set
    m.get_axon_ntff_profile_hook = _get
    sys.modules["antenv.axon_hooks"] = m
    antenv.axon_hooks = m
    try:
        from trn_agent_boot.trn_boot import _ntff_profile_via_ctypes
        m.set_axon_ntff_profile_hook(
            _ntff_profile_via_ctypes("/opt/axon/libaxon_pjrt.so")
        )
    except Exception:
        m.set_axon_ntff_profile_hook(None)


def run(inputs, trace=False):
    """Returns ((logits, idx, z_q), exec_time_ns_or_None)."""
    in_maps, flags = _prep_inputs(inputs)
    nc = _build(*flags)
    if trace:
        _inject_ntff_hook()
    res = run_bass_kernel_spmd(
        nc, in_maps, core_ids=list(range(NCORES)), trace=trace
    )
    return _assemble(res.results), (res.exec_time_ns if trace else None)


def kernel(**inputs):
    out, _ = run(inputs, trace=False)
    return out


# revision 20
# speedup vs baseline: 1.0240x; 1.0240x over previous
"""Trainium2 Bass kernel for nn_Attention_63866163692087 (vq_codebook).

Math (verified against the reference):
  logits[b,n,t] = a * sum_c Qg[bt,c] * K[n,c] + beta[bt]
    where Qg = (hs^T @ Wq + bq) * rep(a*(hs^T @ Wp + bp)),  a = 1/sqrt(d*H),
          K = cb @ Wk + bk  (Wk folded into the Q side: Qk = Qg @ Wk^T,
          logits = Qk @ cb^T + beta,  beta = Qg @ bk)
  idx = argmax_n logits          (softmax is monotonic)
  z_q[b,:,t] = cb[idx] @ Wv + bv (straight-through output is a pure gather)

Sharding: 8 cores, each takes 1024 rows of the flattened (b, t) axis
(core i -> b = i//2, t-half = i%2). No collectives.

Per-core layouts: everything is computed in a "transposed" orientation so
argmax runs along the DVE free axis and z_q comes out channel-major:
  hsT  [C, TL]     (native layout of hidden_states[b])
  QgT/QkT [C, TL]
  logits tiles [t(128), N] -> DMA to logits_loc [TL, N] (host transposes)
  zqT  [C, TL]     (direct concat on host)
"""

import os
import sys
import types

for _p in ("/opt/trn_rl_repo", "/root/.axon_site/_ro/trn_rl_repo"):
    if os.path.isdir(_p) and _p not in sys.path:
        sys.path.insert(0, _p)

import numpy as np
import ml_dtypes

import concourse.bass as bass
import concourse.mybir as mybir
from concourse.tile import TileContext
from concourse.vector_clock import ScopedClock
from concourse.masks import make_identity
from concourse.bass_utils import run_bass_kernel_spmd

B, C, T, N, H = 4, 512, 2048, 4096, 4
D = C // H                     # 128, head dim == one partition tile
P = 128
NCORES = 8
TL = (B * T) // NCORES         # 1024 local (b,t) rows per core
ALPHA = 1.0 / float(np.sqrt(D) * np.sqrt(H))

F32 = mybir.dt.float32
BF16 = mybir.dt.bfloat16
F32R = mybir.dt.float32r
U32 = mybir.dt.uint32
COPY = mybir.ActivationFunctionType.Copy
IDENT = mybir.ActivationFunctionType.Identity

# main-matmul dtype: "f32r" (1 cyc/row) or "f32" (4 cyc/row, exact)
MM_DT = os.environ.get("BASS_VQ_MMDT", "bf3")


def _patch_tail_drain():
    """Spread the kernel-tail drain's frontier waits over single-wait SP nops
    (walrus rejects >4 sync waits on one instruction)."""
    if getattr(TileContext, "_vq_drain_patched", False):
        return

    def _patched_dab(self, tick_clock, wait_clock):
        probe = self.nc.sync.nop()
        wait_clock.add_sem_waits(
            probe.ins, ScopedClock({None: tick_clock.global_clock})
        )
        si = probe.ins.sync_info
        if si is not None and si.on_wait is not None and len(si.on_wait) > 1:
            waits = list(si.on_wait)
            probe.ins.sync_info = mybir.SyncInfo(
                on_wait=waits[:1], on_update=list(si.on_update or [])
            )
            for w in waits[1:]:
                extra = self.nc.sync.nop()
                extra.ins.sync_info = mybir.SyncInfo(on_wait=[w], on_update=[])
        self.nc.sync.drain()
        self.nc.all_engine_barrier()
        assert self.sems is not None
        popped = self.nc._tile_sem_poison_stack.pop()
        assert popped is self._sem_poison
        self.nc.clear_and_free_semaphores(list(self.sems.allocated().values()))
        self.nc.all_engine_barrier()

    TileContext._drain_and_barrier = _patched_dab
    TileContext._vq_drain_patched = True


def _legalize_waits(nc, max_waits=1):
    """Walrus accepts only a limited number of sync waits per instruction
    (fused fp32 matmuls appear to accept just one). Move excess waits onto
    freshly inserted same-engine NOPs immediately before the instruction —
    engines execute in order, so semantics are preserved."""
    k = 0
    for f in nc.m.functions:
        for blk in f.blocks:
            il = blk.instructions
            i = 0
            while i < len(il):
                inst = il[i]
                si = getattr(inst, "sync_info", None)
                eng = getattr(inst, "engine", None)
                if (
                    si is not None
                    and si.on_wait is not None
                    and len(si.on_wait) > max_waits
                    and eng is not None
                    and eng != mybir.EngineType.Unassigned
                ):
                    waits = list(si.on_wait)
                    keep, excess = waits[:max_waits], waits[max_waits:]
                    inst.sync_info = mybir.SyncInfo(
                        on_wait=keep, on_update=list(si.on_update or [])
                    )
                    for w in excess:
                        nop = mybir.InstNoOp(
                            name=f"I-waitfix-{k}", ins=[], outs=[]
                        )
                        k += 1
                        nop.engine = eng
                        nop.sync_info = mybir.SyncInfo(on_wait=[w], on_update=[])
                        il.insert(i, nop)
                        i += 1
                i += 1
    return k


def _build(use_qp_bias, use_bk, use_bv, mm_dt=None):
    """Per-core Bass graph (SPMD across 8 cores).

    mm_dt: 'f32' — exact fp32 matmuls (4 cyc/row) on the logits chain
           'bf3' — bf16 hi/lo 3-product split (3x 1 cyc/row, near-exact)
    """
    if mm_dt is None:
        mm_dt = MM_DT
    _patch_tail_drain()
    nc = bass.Bass()
    E = 5 if use_qp_bias else 4
    CE = E * P
    bf3 = mm_dt == "bf3"
    MDT = BF16 if bf3 else F32
    # (lhs split, rhs split) product passes: x = h + l, drop l*l
    PASSES = [(0, 0), (0, 1), (1, 0)] if bf3 else [(0, 0)]
    NSP = 2 if bf3 else 1

    def dparam(name, shape, dt):
        return nc.declare_dram_parameter(name, shape, dt, isOutput=False)

    if bf3:
        hst_d = [dparam("hsth", [CE, TL], BF16), dparam("hstl", [CE, TL], BF16)]
        cbt_d = [dparam("cbth", [C, N], BF16), dparam("cbtl", [C, N], BF16)]
        wq_d = [dparam("wqh", [CE, C], BF16), dparam("wql", [CE, C], BF16)]
        wp_d = [dparam("wph", [CE, H], BF16), dparam("wpl", [CE, H], BF16)]
        wkt_d = [dparam("wkth", [C, C], BF16), dparam("wktl", [C, C], BF16)]
    else:
        hst_d = [dparam("hst", [CE, TL], F32)]
        cbt_d = [dparam("cbt", [C, N], F32)]
        wq_d = [dparam("wq", [CE, C], F32)]
        wp_d = [dparam("wp", [CE, H], F32)]
        wkt_d = [dparam("wkt", [C, C], F32)]
    cb_d = dparam("cb", [N, C], F32)
    wv_d = dparam("wv", [C, C], BF16)
    if use_bk:
        bk_d = dparam("bk", [C, 1], F32)
    if use_bv:
        bv_d = dparam("bv", [C, 1], F32)

    logits_d = nc.declare_dram_parameter("logits", [TL, N], F32, isOutput=True)
    idx_d = nc.declare_dram_parameter("idx", [TL, 1], U32, isOutput=True)
    zqt_d = nc.declare_dram_parameter("zqt", [C, TL], F32, isOutput=True)

    with TileContext(nc) as tc:
        with (
            tc.tile_pool(name="const", bufs=1) as const,
            tc.tile_pool(name="work", bufs=1) as work,
            tc.tile_pool(name="gates", bufs=2) as gpool,
            tc.tile_pool(name="lrow", bufs=2) as lpool,
            tc.tile_pool(name="small", bufs=2) as spool,
            tc.tile_pool(name="zq", bufs=2) as zpool,
            tc.tile_pool(name="psmm", bufs=4, space="PSUM") as psmm,
            tc.tile_pool(name="pstp", bufs=2, space="PSUM") as pstp,
            tc.tile_pool(name="pssm", bufs=2, space="PSUM") as pssm,
        ):
            # ---------------- loads ----------------
            hst, wq, wp4, wkt, cbt = {}, {}, {}, {}, {}
            for sp in range(NSP):
                hst[sp] = [const.tile([P, TL], MDT, name=f"hst{sp}_{e}") for e in range(E)]
                wq[sp] = [const.tile([P, C], MDT, name=f"wq{sp}_{e}") for e in range(E)]
                wp4[sp] = [const.tile([P, H], MDT, name=f"wp4{sp}_{e}") for e in range(E)]
                wkt[sp] = [const.tile([P, C], MDT, name=f"wkt{sp}_{c}") for c in range(4)]
                cbt[sp] = [const.tile([P, N], MDT, name=f"cbt{sp}_{e}") for e in range(4)]
            for sp in range(NSP):
                for e in range(E):
                    nc.sync.dma_start(hst[sp][e][:], hst_d[sp][e * P:(e + 1) * P, :])
                    nc.sync.dma_start(wq[sp][e][:], wq_d[sp][e * P:(e + 1) * P, :])
                    nc.sync.dma_start(wp4[sp][e][:], wp_d[sp][e * P:(e + 1) * P, :])
                for c4 in range(4):
                    nc.sync.dma_start(wkt[sp][c4][:], wkt_d[sp][c4 * P:(c4 + 1) * P, :])
            # one big DMA per cbt tile (many small piece-DMAs serialize on
            # the issue queue and starve stage B)
            for sp in range(NSP):
                for c4 in range(4):
                    nc.sync.dma_start(
                        cbt[sp][c4][:], cbt_d[sp][c4 * P:(c4 + 1) * P, :]
                    )
            wv = [const.tile([P, C], BF16, name=f"wv{c}") for c in range(4)]
            for c4 in range(4):
                nc.sync.dma_start(wv[c4][:], wv_d[c4 * P:(c4 + 1) * P, :])
            ident = const.tile([P, P], F32, name="ident")
            make_identity(nc, ident[:])
            identb = const.tile([P, P], BF16, name="identb")
            nc.vector.tensor_copy(identb[:], ident[:])
            if use_bk:
                bks = const.tile([P, 4], F32, name="bks")
                nc.sync.dma_start(bks[:], bk_d.rearrange("(c p) o -> p (c o)", p=P))
                if bf3:
                    # beta matmul operands must match qgt dtype (bf16);
                    # bk's low bf16 bits are dropped (bk==0 in practice)
                    bks_m = const.tile([P, 4], BF16, name="bksm")
                    nc.vector.tensor_copy(bks_m[:], bks[:])
                else:
                    bks_m = bks
            if use_bv:
                bvs = const.tile([P, 4], F32, name="bvs")
                nc.sync.dma_start(bvs[:], bv_d.rearrange("(c p) o -> p (c o)", p=P))

            # device-split QgT / QkT (h/l in bf16 when bf3, else plain f32)
            qgt = {sp: [work.tile([P, TL], MDT, name=f"qgt{sp}_{c}") for c in range(4)]
                   for sp in range(NSP)}
            qkt = {sp: [work.tile([P, TL], MDT, name=f"qkt{sp}_{c}") for c in range(4)]
                   for sp in range(NSP)}
            cbgt = [work.tile([P, TL], BF16, name=f"cbgt{c}") for c in range(4)]
            g4a = work.tile([P, 8 * H], F32, name="g4a")
            if use_bk:
                beta = work.tile([P, 8], F32, name="beta")

            # ------- gates: G[t, h] per t-chunk, then broadcast-transpose ----
            for i in range(8):
                g4p = pssm.tile([P, H], F32, tag="pss", name="g4p")
                nmm = len(PASSES) * E
                k = 0
                for (a, b) in PASSES:
                    for e in range(E):
                        nc.tensor.matmul(
                            g4p[:], lhsT=hst[b][e][:, i * P:(i + 1) * P],
                            rhs=wp4[a][e][:],
                            start=(k == 0), stop=(k == nmm - 1),
                        )
                        k += 1
                nc.scalar.activation(g4a[:, i * H:(i + 1) * H], g4p[:], COPY)

            # ---------------- stage A: QgT / beta / QkT ----------------
            for s in range(2):
                tsl = slice(s * 512, (s + 1) * 512)
                gsb = []
                for h in range(4):
                    grp = psmm.tile([P, 512], F32, tag="ps", name="grp")
                    for j in range(4):
                        i = s * 4 + j
                        nc.tensor.transpose(
                            grp[:, j * P:(j + 1) * P],
                            g4a[:, i * H + h:i * H + h + 1].to_broadcast([P, P]),
                            ident[:],
                        )
                    g = gpool.tile([P, 512], F32, tag="g", name="g")
                    nc.scalar.activation(g[:], grp[:], COPY)
                    gsb.append(g)
                for c4 in range(4):
                    qp = psmm.tile([P, 512], F32, tag="ps", name="qp")
                    nmm = len(PASSES) * E
                    k = 0
                    for (a, b) in PASSES:
                        for e in range(E):
                            nc.tensor.matmul(
                                qp[:], lhsT=wq[a][e][:, c4 * P:(c4 + 1) * P],
                                rhs=hst[b][e][:, tsl],
                                start=(k == 0), stop=(k == nmm - 1),
                            )
                            k += 1
                    if bf3:
                        qg32 = gpool.tile([P, 512], F32, tag="qg32", name="qg32")
                        nc.vector.tensor_mul(qg32[:], qp[:], gsb[c4][:])
                        nc.scalar.activation(qgt[0][c4][:, tsl], qg32[:], COPY)
                        nc.vector.tensor_sub(
                            qgt[1][c4][:, tsl], qg32[:], qgt[0][c4][:, tsl]
                        )
                    else:
                        nc.vector.tensor_mul(qgt[0][c4][:, tsl], qp[:], gsb[c4][:])
                if use_bk:
                    for i in range(s * 4, s * 4 + 4):
                        bp = pssm.tile([P, H], F32, tag="pss", name="bp")
                        nmm = 4 * NSP
                        k = 0
                        for sp in range(NSP):
                            for c4 in range(4):
                                nc.tensor.matmul(
                                    bp[:, 0:1],
                                    lhsT=qgt[sp][c4][:, i * P:(i + 1) * P],
                                    rhs=bks_m[:, c4:c4 + 1],
                                    start=(k == 0), stop=(k == nmm - 1),
                                )
                                k += 1
                        nc.scalar.activation(beta[:, i:i + 1], bp[:, 0:1], COPY)
                for e4 in range(4):
                    kp = psmm.tile([P, 512], F32, tag="ps", name="kp")
                    nmm = len(PASSES) * 4
                    k = 0
                    for (a, b) in PASSES:
                        for c4 in range(4):
                            nc.tensor.matmul(
                                kp[:], lhsT=wkt[a][c4][:, e4 * P:(e4 + 1) * P],
                                rhs=qgt[b][c4][:, tsl],
                                start=(k == 0), stop=(k == nmm - 1),
                            )
                            k += 1
                    nc.scalar.activation(qkt[0][e4][:, tsl], kp[:], COPY)
                    if bf3:
                        nc.vector.tensor_sub(
                            qkt[1][e4][:, tsl], kp[:], qkt[0][e4][:, tsl]
                        )

            # ---------------- stage C helper: z_q for one t-half -------------
            def stage_c(s):
                tsl_c = slice(s * 512, (s + 1) * 512)
                for c4 in range(4):
                    zp = psmm.tile([P, 512], F32, tag="ps", name="zp")
                    for e4 in range(4):
                        nc.tensor.matmul(
                            zp[:], lhsT=wv[e4][:, c4 * P:(c4 + 1) * P],
                            rhs=cbgt[e4][:, tsl_c],
                            start=(e4 == 0), stop=(e4 == 3),
                        )
                    z = zpool.tile([P, 512], F32, tag="z", name="z")
                    if use_bv:
                        nc.scalar.activation(z[:], zp[:], IDENT, bias=bvs[:, c4:c4 + 1])
                    else:
                        nc.scalar.activation(z[:], zp[:], COPY)
                    nc.sync.dma_start(zqt_d[c4 * P:(c4 + 1) * P, tsl_c], z[:])

            # ---------------- stage B: logits, argmax, gather ----------------
            cbgs = []
            for i in range(8):
                tsl = slice(i * P, (i + 1) * P)
                lsb = lpool.tile([P, N], F32, tag="l", name="l")
                for n8 in range(8):
                    nsl = slice(n8 * 512, (n8 + 1) * 512)
                    lp = psmm.tile([P, 512], F32, tag="ps", name="lp")
                    nmm = len(PASSES) * 4
                    k = 0
                    for (a, b) in PASSES:
                        for e4 in range(4):
                            nc.tensor.matmul(
                                lp[:], lhsT=qkt[a][e4][:, tsl],
                                rhs=cbt[b][e4][:, nsl],
                                start=(k == 0), stop=(k == nmm - 1),
                            )
                            k += 1
                    if use_bk:
                        nc.scalar.activation(
                            lsb[:, nsl], lp[:], IDENT, bias=beta[:, i:i + 1]
                        )
                    else:
                        nc.scalar.activation(lsb[:, nsl], lp[:], COPY)
                nc.sync.dma_start(logits_d[tsl, :], lsb[:])
                m8 = spool.tile([P, 8], F32, tag="m8", name="m8")
                i8 = spool.tile([P, 8], U32, tag="i8", name="i8")
                nc.vector.max(out=m8[:], in_=lsb[:])
                nc.vector.max_index(out=i8[:], in_max=m8[:], in_values=lsb[:])
                nc.sync.dma_start(idx_d[tsl, :], i8[:, 0:1])
                cbg = spool.tile([P, C], F32, tag="cbg", name="cbg")
                nc.gpsimd.indirect_dma_start(
                    out=cbg[:], out_offset=None, in_=cb_d[:, :],
                    in_offset=bass.IndirectOffsetOnAxis(ap=i8[:, 0:1], axis=0),
                )
                cbgb = work.tile([P, C], BF16, name=f"cbgb{i}")
                nc.vector.tensor_copy(cbgb[:], cbg[:])
                cbgs.append(cbgb)

            # ---- deferred z_q tail: keep gather->transpose->zmm off the
            # per-chunk PE critical path (PE is in-order; a transpose waiting
            # on chunk i's gather would block chunk i+1's matmuls)
            for i in range(8):
                tsl = slice(i * P, (i + 1) * P)
                for e4 in range(4):
                    tp = pstp.tile([P, P], BF16, tag="pst", name="tp")
                    nc.tensor.transpose(tp[:], cbgs[i][:, e4 * P:(e4 + 1) * P], identb[:])
                    nc.scalar.activation(cbgt[e4][:, tsl], tp[:], COPY)
                if i == 3:
                    stage_c(0)
                elif i == 7:
                    stage_c(1)



    _legalize_waits(nc)
    return nc


def _prep_inputs(inputs):
    """Host-side prep shared by all cores + per-core hs slices."""
    hs = np.ascontiguousarray(np.asarray(inputs["hidden_states"], dtype=np.float32))
    cb = np.ascontiguousarray(
        np.asarray(inputs["codebook_hidden_states"], dtype=np.float32)
    )
    Wq = np.asarray(inputs["Wq"], dtype=np.float32)
    bq = np.asarray(inputs["bq"], dtype=np.float32)
    Wk = np.asarray(inputs["Wk"], dtype=np.float32)
    bk = np.asarray(inputs["bk"], dtype=np.float32)
    Wv = np.asarray(inputs["Wv"], dtype=np.float32)
    bv = np.asarray(inputs["bv"], dtype=np.float32)
    Wp = np.asarray(inputs["Wp"], dtype=np.float32)
    bp = np.asarray(inputs["bp"], dtype=np.float32)

    use_qp_bias = bool(np.any(bq != 0) or np.any(bp != 0))
    use_bk = bool(np.any(bk != 0))
    use_bv = bool(np.any(bv != 0))

    E = 5 if use_qp_bias else 4
    CE = E * P

    wq_eff = np.zeros((CE, C), np.float32)
    wq_eff[:C] = Wq
    wp_eff = np.zeros((CE, H), np.float32)
    wp_eff[:C] = ALPHA * Wp
    if use_qp_bias:
        wq_eff[C] = bq
        wp_eff[C] = ALPHA * bp

    cbt = np.ascontiguousarray(cb.T)
    wkt = np.ascontiguousarray(Wk.T)
    wv_bf = Wv.astype(ml_dtypes.bfloat16)
    bf3 = MM_DT == "bf3"

    def split_bf(x):
        h = x.astype(ml_dtypes.bfloat16)
        l = (x - h.astype(np.float32)).astype(ml_dtypes.bfloat16)
        return h, l

    common = {"cb": cb, "wv": wv_bf}
    if bf3:
        for nm, arr in (("cbt", cbt), ("wq", wq_eff), ("wp", wp_eff),
                        ("wkt", wkt)):
            h, l = split_bf(arr)
            common[nm + "h"] = h
            common[nm + "l"] = l
    else:
        common.update({"cbt": cbt, "wq": wq_eff, "wp": wp_eff, "wkt": wkt})
    if use_bk:
        common["bk"] = bk.reshape(C, 1)
    if use_bv:
        common["bv"] = bv.reshape(C, 1)

    in_maps = []
    for core in range(NCORES):
        b, half = core // 2, core % 2
        hstc = np.zeros((CE, TL), np.float32)
        hstc[:C] = hs[b][:, half * TL:(half + 1) * TL]
        if use_qp_bias:
            hstc[C] = 1.0
        if bf3:
            h, l = split_bf(hstc)
            in_maps.append({"hsth": h, "hstl": l, **common})
        else:
            in_maps.append({"hst": hstc, **common})
    return in_maps, (use_qp_bias, use_bk, use_bv)


def _assemble(results):
    logits = np.empty((B, N, T), np.float32)
    idx = np.empty((B, 1, T), np.int32)
    z_q = np.empty((B, C, T), np.float32)
    for core in range(NCORES):
        b, half = core // 2, core % 2
        tsl = slice(half * TL, (half + 1) * TL)
        r = results[core]
        logits[b][:, tsl] = r["logits"].T
        idx[b, 0, tsl] = r["idx"][:, 0].astype(np.int64).astype(np.int32)
        z_q[b][:, tsl] = r["zqt"]
    return logits, idx, z_q


def _inject_ntff_hook():
    import antenv
    if "antenv.axon_hooks" in sys.modules:
        return
    m = types.ModuleType("antenv.axon_hooks")
    m._hook = None
    def _set(h):
        m._hook = h
    def _get():
        return m._hook
    m.set_axon_ntff_profile_hook = _set
    m.get_axon_ntff_profile_hook = _get
    sys.modules["antenv.axon_hooks"] = m
    antenv.axon_hooks = m
    try:
        from trn_agent_boot.trn_boot import _ntff_profile_via_ctypes
        m.set_axon_ntff_profile_hook(
            _ntff_profile_via_ctypes("/opt/axon/libaxon_pjrt.so")
        )
    except Exception:
        m.set_axon_ntff_profile_hook(None)


def run(inputs, trace=False):
    """Returns ((logits, idx, z_q), exec_time_ns_or_None)."""
    in_maps, flags = _prep_inputs(inputs)
    nc = _build(*flags)
    if trace:
        _inject_ntff_hook()
    res = run_bass_kernel_spmd(
        nc, in_maps, core_ids=list(range(NCORES)), trace=trace
    )
    return _assemble(res.results), (res.exec_time_ns if trace else None)


def kernel(**inputs):
    out, _ = run(inputs, trace=False)
    return out


# revision 21
# speedup vs baseline: 1.0598x; 1.0350x over previous
"""Trainium2 Bass kernel for nn_Attention_63866163692087 (vq_codebook).

Math (verified against the reference):
  logits[b,n,t] = a * sum_c Qg[bt,c] * K[n,c] + beta[bt]
    where Qg = (hs^T @ Wq + bq) * rep(a*(hs^T @ Wp + bp)),  a = 1/sqrt(d*H),
          K = cb @ Wk + bk  (Wk folded into the Q side: Qk = Qg @ Wk^T,
          logits = Qk @ cb^T + beta,  beta = Qg @ bk)
  idx = argmax_n logits          (softmax is monotonic)
  z_q[b,:,t] = cb[idx] @ Wv + bv (straight-through output is a pure gather)

Sharding: 8 cores, each takes 1024 rows of the flattened (b, t) axis
(core i -> b = i//2, t-half = i%2). No collectives.

Per-core layouts: everything is computed in a "transposed" orientation so
argmax runs along the DVE free axis and z_q comes out channel-major:
  hsT  [C, TL]     (native layout of hidden_states[b])
  QgT/QkT [C, TL]
  logits tiles [t(128), N] -> DMA to logits_loc [TL, N] (host transposes)
  zqT  [C, TL]     (direct concat on host)
"""

import os
import sys
import types

for _p in ("/opt/trn_rl_repo", "/root/.axon_site/_ro/trn_rl_repo"):
    if os.path.isdir(_p) and _p not in sys.path:
        sys.path.insert(0, _p)

import numpy as np
import ml_dtypes

import concourse.bass as bass
import concourse.mybir as mybir
from concourse.tile import TileContext
from concourse.vector_clock import ScopedClock
from concourse.masks import make_identity
from concourse.bass_utils import run_bass_kernel_spmd

B, C, T, N, H = 4, 512, 2048, 4096, 4
D = C // H                     # 128, head dim == one partition tile
P = 128
NCORES = 8
TL = (B * T) // NCORES         # 1024 local (b,t) rows per core
ALPHA = 1.0 / float(np.sqrt(D) * np.sqrt(H))

F32 = mybir.dt.float32
BF16 = mybir.dt.bfloat16
F32R = mybir.dt.float32r
U32 = mybir.dt.uint32
COPY = mybir.ActivationFunctionType.Copy
IDENT = mybir.ActivationFunctionType.Identity

# main-matmul dtype: "f32r" (1 cyc/row) or "f32" (4 cyc/row, exact)
MM_DT = os.environ.get("BASS_VQ_MMDT", "bf3")


def _patch_tail_drain():
    """Spread the kernel-tail drain's frontier waits over single-wait SP nops
    (walrus rejects >4 sync waits on one instruction)."""
    if getattr(TileContext, "_vq_drain_patched", False):
        return

    def _patched_dab(self, tick_clock, wait_clock):
        probe = self.nc.sync.nop()
        wait_clock.add_sem_waits(
            probe.ins, ScopedClock({None: tick_clock.global_clock})
        )
        si = probe.ins.sync_info
        if si is not None and si.on_wait is not None and len(si.on_wait) > 1:
            waits = list(si.on_wait)
            probe.ins.sync_info = mybir.SyncInfo(
                on_wait=waits[:1], on_update=list(si.on_update or [])
            )
            for w in waits[1:]:
                extra = self.nc.sync.nop()
                extra.ins.sync_info = mybir.SyncInfo(on_wait=[w], on_update=[])
        self.nc.sync.drain()
        self.nc.all_engine_barrier()
        assert self.sems is not None
        popped = self.nc._tile_sem_poison_stack.pop()
        assert popped is self._sem_poison
        self.nc.clear_and_free_semaphores(list(self.sems.allocated().values()))
        self.nc.all_engine_barrier()

    TileContext._drain_and_barrier = _patched_dab
    TileContext._vq_drain_patched = True


def _legalize_waits(nc, max_waits=1):
    """Walrus accepts only a limited number of sync waits per instruction
    (fused fp32 matmuls appear to accept just one). Move excess waits onto
    freshly inserted same-engine NOPs immediately before the instruction —
    engines execute in order, so semantics are preserved."""
    k = 0
    for f in nc.m.functions:
        for blk in f.blocks:
            il = blk.instructions
            i = 0
            while i < len(il):
                inst = il[i]
                si = getattr(inst, "sync_info", None)
                eng = getattr(inst, "engine", None)
                if (
                    si is not None
                    and si.on_wait is not None
                    and len(si.on_wait) > max_waits
                    and eng is not None
                    and eng != mybir.EngineType.Unassigned
                ):
                    waits = list(si.on_wait)
                    keep, excess = waits[:max_waits], waits[max_waits:]
                    inst.sync_info = mybir.SyncInfo(
                        on_wait=keep, on_update=list(si.on_update or [])
                    )
                    for w in excess:
                        nop = mybir.InstNoOp(
                            name=f"I-waitfix-{k}", ins=[], outs=[]
                        )
                        k += 1
                        nop.engine = eng
                        nop.sync_info = mybir.SyncInfo(on_wait=[w], on_update=[])
                        il.insert(i, nop)
                        i += 1
                i += 1
    return k


def _build(use_qp_bias, use_bk, use_bv, mm_dt=None):
    """Per-core Bass graph (SPMD across 8 cores).

    mm_dt: 'f32' — exact fp32 matmuls (4 cyc/row) on the logits chain
           'bf3' — bf16 hi/lo 3-product split (3x 1 cyc/row, near-exact)
    """
    if mm_dt is None:
        mm_dt = MM_DT
    _patch_tail_drain()
    nc = bass.Bass()
    E = 5 if use_qp_bias else 4
    CE = E * P
    bf3 = mm_dt == "bf3"
    MDT = BF16 if bf3 else F32
    # (lhs split, rhs split) product passes: x = h + l, drop l*l
    PASSES = [(0, 0), (0, 1), (1, 0)] if bf3 else [(0, 0)]
    NSP = 2 if bf3 else 1

    def dparam(name, shape, dt):
        return nc.declare_dram_parameter(name, shape, dt, isOutput=False)

    if bf3:
        # h and l halves concatenated along the free axis: one DMA per tile
        # pair (separate small loads serialize on the issue queue at startup)
        hst_d = dparam("hsthl", [CE, 2 * TL], BF16)
        cbt_d = dparam("cbthl", [C, 2 * N], BF16)
        wq_d = dparam("wqhl", [CE, 2 * C], BF16)
        wp_d = dparam("wphl", [CE, 2 * H], BF16)
        wkt_d = dparam("wkthl", [C, 2 * C], BF16)
    else:
        hst_d = [dparam("hst", [CE, TL], F32)]
        cbt_d = [dparam("cbt", [C, N], F32)]
        wq_d = [dparam("wq", [CE, C], F32)]
        wp_d = [dparam("wp", [CE, H], F32)]
        wkt_d = [dparam("wkt", [C, C], F32)]
    cb_d = dparam("cb", [N, C], F32)
    wv_d = dparam("wv", [C, C], BF16)
    if use_bk:
        bk_d = dparam("bk", [C, 1], F32)
    if use_bv:
        bv_d = dparam("bv", [C, 1], F32)

    logits_d = nc.declare_dram_parameter("logits", [TL, N], F32, isOutput=True)
    idx_d = nc.declare_dram_parameter("idx", [TL, 1], U32, isOutput=True)
    zqt_d = nc.declare_dram_parameter("zqt", [C, TL], F32, isOutput=True)

    with TileContext(nc) as tc:
        with (
            tc.tile_pool(name="const", bufs=1) as const,
            tc.tile_pool(name="work", bufs=1) as work,
            tc.tile_pool(name="gates", bufs=2) as gpool,
            tc.tile_pool(name="lrow", bufs=2) as lpool,
            tc.tile_pool(name="small", bufs=2) as spool,
            tc.tile_pool(name="zq", bufs=2) as zpool,
            tc.tile_pool(name="psmm", bufs=4, space="PSUM") as psmm,
            tc.tile_pool(name="pstp", bufs=2, space="PSUM") as pstp,
            tc.tile_pool(name="pssm", bufs=2, space="PSUM") as pssm,
        ):
            # ---------------- loads ----------------
            hst, wq, wp4, wkt, cbt = {}, {}, {}, {}, {}
            if bf3:
                hst_t = [const.tile([P, 2 * TL], BF16, name=f"hst_{e}") for e in range(E)]
                wq_t = [const.tile([P, 2 * C], BF16, name=f"wq_{e}") for e in range(E)]
                wp_t = [const.tile([P, 2 * H], BF16, name=f"wp_{e}") for e in range(E)]
                wkt_t = [const.tile([P, 2 * C], BF16, name=f"wkt_{c}") for c in range(4)]
                cbt_t = [const.tile([P, 2 * N], BF16, name=f"cbt_{e}") for e in range(4)]
                for e in range(E):
                    nc.sync.dma_start(hst_t[e][:], hst_d[e * P:(e + 1) * P, :])
                    nc.sync.dma_start(wq_t[e][:], wq_d[e * P:(e + 1) * P, :])
                    nc.sync.dma_start(wp_t[e][:], wp_d[e * P:(e + 1) * P, :])
                for c4 in range(4):
                    nc.sync.dma_start(wkt_t[c4][:], wkt_d[c4 * P:(c4 + 1) * P, :])
                for c4 in range(4):
                    nc.sync.dma_start(cbt_t[c4][:], cbt_d[c4 * P:(c4 + 1) * P, :])
                for sp in range(NSP):
                    hst[sp] = [hst_t[e][:, sp * TL:(sp + 1) * TL] for e in range(E)]
                    wq[sp] = [wq_t[e][:, sp * C:(sp + 1) * C] for e in range(E)]
                    wp4[sp] = [wp_t[e][:, sp * H:(sp + 1) * H] for e in range(E)]
                    wkt[sp] = [wkt_t[c][:, sp * C:(sp + 1) * C] for c in range(4)]
                    cbt[sp] = [cbt_t[c][:, sp * N:(sp + 1) * N] for c in range(4)]
            else:
                for sp in range(NSP):
                    hst[sp] = [const.tile([P, TL], MDT, name=f"hst{sp}_{e}") for e in range(E)]
                    wq[sp] = [const.tile([P, C], MDT, name=f"wq{sp}_{e}") for e in range(E)]
                    wp4[sp] = [const.tile([P, H], MDT, name=f"wp4{sp}_{e}") for e in range(E)]
                    wkt[sp] = [const.tile([P, C], MDT, name=f"wkt{sp}_{c}") for c in range(4)]
                    cbt[sp] = [const.tile([P, N], MDT, name=f"cbt{sp}_{e}") for e in range(4)]
                    for e in range(E):
                        nc.sync.dma_start(hst[sp][e][:], hst_d[sp][e * P:(e + 1) * P, :])
                        nc.sync.dma_start(wq[sp][e][:], wq_d[sp][e * P:(e + 1) * P, :])
                        nc.sync.dma_start(wp4[sp][e][:], wp_d[sp][e * P:(e + 1) * P, :])
                    for c4 in range(4):
                        nc.sync.dma_start(wkt[sp][c4][:], wkt_d[sp][c4 * P:(c4 + 1) * P, :])
                        nc.sync.dma_start(cbt[sp][c4][:], cbt_d[sp][c4 * P:(c4 + 1) * P, :])
            wv = [const.tile([P, C], BF16, name=f"wv{c}") for c in range(4)]
            for c4 in range(4):
                nc.sync.dma_start(wv[c4][:], wv_d[c4 * P:(c4 + 1) * P, :])
            ident = const.tile([P, P], F32, name="ident")
            make_identity(nc, ident[:])
            identb = const.tile([P, P], BF16, name="identb")
            nc.vector.tensor_copy(identb[:], ident[:])
            if use_bk:
                bks = const.tile([P, 4], F32, name="bks")
                nc.sync.dma_start(bks[:], bk_d.rearrange("(c p) o -> p (c o)", p=P))
                if bf3:
                    # beta matmul operands must match qgt dtype (bf16);
                    # bk's low bf16 bits are dropped (bk==0 in practice)
                    bks_m = const.tile([P, 4], BF16, name="bksm")
                    nc.vector.tensor_copy(bks_m[:], bks[:])
                else:
                    bks_m = bks
            if use_bv:
                bvs = const.tile([P, 4], F32, name="bvs")
                nc.sync.dma_start(bvs[:], bv_d.rearrange("(c p) o -> p (c o)", p=P))

            # device-split QgT / QkT (h/l in bf16 when bf3, else plain f32)
            qgt = {sp: [work.tile([P, TL], MDT, name=f"qgt{sp}_{c}") for c in range(4)]
                   for sp in range(NSP)}
            qkt = {sp: [work.tile([P, TL], MDT, name=f"qkt{sp}_{c}") for c in range(4)]
                   for sp in range(NSP)}
            cbgt = [work.tile([P, TL], BF16, name=f"cbgt{c}") for c in range(4)]
            g4a = work.tile([P, 8 * H], F32, name="g4a")
            if use_bk:
                beta = work.tile([P, 8], F32, name="beta")

            # ------- gates: G[t, h] per t-chunk, then broadcast-transpose ----
            for i in range(8):
                g4p = pssm.tile([P, H], F32, tag="pss", name="g4p")
                nmm = len(PASSES) * E
                k = 0
                for (a, b) in PASSES:
                    for e in range(E):
                        nc.tensor.matmul(
                            g4p[:], lhsT=hst[b][e][:, i * P:(i + 1) * P],
                            rhs=wp4[a][e][:],
                            start=(k == 0), stop=(k == nmm - 1),
                        )
                        k += 1
                nc.scalar.activation(g4a[:, i * H:(i + 1) * H], g4p[:], COPY)

            # ---------------- stage A: QgT / beta / QkT ----------------
            for s in range(2):
                tsl = slice(s * 512, (s + 1) * 512)
                gsb = []
                for h in range(4):
                    grp = psmm.tile([P, 512], F32, tag="ps", name="grp")
                    for j in range(4):
                        i = s * 4 + j
                        nc.tensor.transpose(
                            grp[:, j * P:(j + 1) * P],
                            g4a[:, i * H + h:i * H + h + 1].to_broadcast([P, P]),
                            ident[:],
                        )
                    g = gpool.tile([P, 512], F32, tag="g", name="g")
                    nc.scalar.activation(g[:], grp[:], COPY)
                    gsb.append(g)
                for c4 in range(4):
                    qp = psmm.tile([P, 512], F32, tag="ps", name="qp")
                    nmm = len(PASSES) * E
                    k = 0
                    for (a, b) in PASSES:
                        for e in range(E):
                            nc.tensor.matmul(
                                qp[:], lhsT=wq[a][e][:, c4 * P:(c4 + 1) * P],
                                rhs=hst[b][e][:, tsl],
                                start=(k == 0), stop=(k == nmm - 1),
                            )
                            k += 1
                    if bf3:
                        qg32 = gpool.tile([P, 512], F32, tag="qg32", name="qg32")
                        nc.vector.tensor_mul(qg32[:], qp[:], gsb[c4][:])
                        nc.scalar.activation(qgt[0][c4][:, tsl], qg32[:], COPY)
                        nc.vector.tensor_sub(
                            qgt[1][c4][:, tsl], qg32[:], qgt[0][c4][:, tsl]
                        )
                    else:
                        nc.vector.tensor_mul(qgt[0][c4][:, tsl], qp[:], gsb[c4][:])
                if use_bk:
                    for i in range(s * 4, s * 4 + 4):
                        bp = pssm.tile([P, H], F32, tag="pss", name="bp")
                        nmm = 4 * NSP
                        k = 0
                        for sp in range(NSP):
                            for c4 in range(4):
                                nc.tensor.matmul(
                                    bp[:, 0:1],
                                    lhsT=qgt[sp][c4][:, i * P:(i + 1) * P],
                                    rhs=bks_m[:, c4:c4 + 1],
                                    start=(k == 0), stop=(k == nmm - 1),
                                )
                                k += 1
                        nc.scalar.activation(beta[:, i:i + 1], bp[:, 0:1], COPY)
                for e4 in range(4):
                    kp = psmm.tile([P, 512], F32, tag="ps", name="kp")
                    nmm = len(PASSES) * 4
                    k = 0
                    for (a, b) in PASSES:
                        for c4 in range(4):
                            nc.tensor.matmul(
                                kp[:], lhsT=wkt[a][c4][:, e4 * P:(e4 + 1) * P],
                                rhs=qgt[b][c4][:, tsl],
                                start=(k == 0), stop=(k == nmm - 1),
                            )
                            k += 1
                    nc.scalar.activation(qkt[0][e4][:, tsl], kp[:], COPY)
                    if bf3:
                        nc.vector.tensor_sub(
                            qkt[1][e4][:, tsl], kp[:], qkt[0][e4][:, tsl]
                        )

            # ---------------- stage C helper: z_q for one t-half -------------
            def stage_c(s):
                tsl_c = slice(s * 512, (s + 1) * 512)
                for c4 in range(4):
                    zp = psmm.tile([P, 512], F32, tag="ps", name="zp")
                    for e4 in range(4):
                        nc.tensor.matmul(
                            zp[:], lhsT=wv[e4][:, c4 * P:(c4 + 1) * P],
                            rhs=cbgt[e4][:, tsl_c],
                            start=(e4 == 0), stop=(e4 == 3),
                        )
                    z = zpool.tile([P, 512], F32, tag="z", name="z")
                    if use_bv:
                        nc.scalar.activation(z[:], zp[:], IDENT, bias=bvs[:, c4:c4 + 1])
                    else:
                        nc.scalar.activation(z[:], zp[:], COPY)
                    nc.sync.dma_start(zqt_d[c4 * P:(c4 + 1) * P, tsl_c], z[:])

            # ---------------- stage B: logits, argmax, gather ----------------
            cbgs = []
            for i in range(8):
                tsl = slice(i * P, (i + 1) * P)
                lsb = lpool.tile([P, N], F32, tag="l", name="l")
                for n8 in range(8):
                    nsl = slice(n8 * 512, (n8 + 1) * 512)
                    lp = psmm.tile([P, 512], F32, tag="ps", name="lp")
                    nmm = len(PASSES) * 4
                    k = 0
                    for (a, b) in PASSES:
                        for e4 in range(4):
                            nc.tensor.matmul(
                                lp[:], lhsT=qkt[a][e4][:, tsl],
                                rhs=cbt[b][e4][:, nsl],
                                start=(k == 0), stop=(k == nmm - 1),
                            )
                            k += 1
                    if use_bk:
                        nc.scalar.activation(
                            lsb[:, nsl], lp[:], IDENT, bias=beta[:, i:i + 1]
                        )
                    else:
                        nc.scalar.activation(lsb[:, nsl], lp[:], COPY)
                nc.sync.dma_start(logits_d[tsl, :], lsb[:])
                m8 = spool.tile([P, 8], F32, tag="m8", name="m8")
                i8 = spool.tile([P, 8], U32, tag="i8", name="i8")
                nc.vector.max(out=m8[:], in_=lsb[:])
                nc.vector.max_index(out=i8[:], in_max=m8[:], in_values=lsb[:])
                nc.sync.dma_start(idx_d[tsl, :], i8[:, 0:1])
                cbg = spool.tile([P, C], F32, tag="cbg", name="cbg")
                nc.gpsimd.indirect_dma_start(
                    out=cbg[:], out_offset=None, in_=cb_d[:, :],
                    in_offset=bass.IndirectOffsetOnAxis(ap=i8[:, 0:1], axis=0),
                )
                cbgb = work.tile([P, C], BF16, name=f"cbgb{i}")
                nc.vector.tensor_copy(cbgb[:], cbg[:])
                cbgs.append(cbgb)

            # ---- deferred z_q tail: keep gather->transpose->zmm off the
            # per-chunk PE critical path (PE is in-order; a transpose waiting
            # on chunk i's gather would block chunk i+1's matmuls)
            for i in range(8):
                tsl = slice(i * P, (i + 1) * P)
                for e4 in range(4):
                    tp = pstp.tile([P, P], BF16, tag="pst", name="tp")
                    nc.tensor.transpose(tp[:], cbgs[i][:, e4 * P:(e4 + 1) * P], identb[:])
                    nc.scalar.activation(cbgt[e4][:, tsl], tp[:], COPY)
                if i == 3:
                    stage_c(0)
                elif i == 7:
                    stage_c(1)



    _legalize_waits(nc)
    return nc


def _prep_inputs(inputs):
    """Host-side prep shared by all cores + per-core hs slices."""
    hs = np.ascontiguousarray(np.asarray(inputs["hidden_states"], dtype=np.float32))
    cb = np.ascontiguousarray(
        np.asarray(inputs["codebook_hidden_states"], dtype=np.float32)
    )
    Wq = np.asarray(inputs["Wq"], dtype=np.float32)
    bq = np.asarray(inputs["bq"], dtype=np.float32)
    Wk = np.asarray(inputs["Wk"], dtype=np.float32)
    bk = np.asarray(inputs["bk"], dtype=np.float32)
    Wv = np.asarray(inputs["Wv"], dtype=np.float32)
    bv = np.asarray(inputs["bv"], dtype=np.float32)
    Wp = np.asarray(inputs["Wp"], dtype=np.float32)
    bp = np.asarray(inputs["bp"], dtype=np.float32)

    use_qp_bias = bool(np.any(bq != 0) or np.any(bp != 0))
    use_bk = bool(np.any(bk != 0))
    use_bv = bool(np.any(bv != 0))

    E = 5 if use_qp_bias else 4
    CE = E * P

    wq_eff = np.zeros((CE, C), np.float32)
    wq_eff[:C] = Wq
    wp_eff = np.zeros((CE, H), np.float32)
    wp_eff[:C] = ALPHA * Wp
    if use_qp_bias:
        wq_eff[C] = bq
        wp_eff[C] = ALPHA * bp

    cbt = np.ascontiguousarray(cb.T)
    wkt = np.ascontiguousarray(Wk.T)
    wv_bf = Wv.astype(ml_dtypes.bfloat16)
    bf3 = MM_DT == "bf3"

    def split_bf(x):
        h = x.astype(ml_dtypes.bfloat16)
        l = (x - h.astype(np.float32)).astype(ml_dtypes.bfloat16)
        return h, l

    common = {"cb": cb, "wv": wv_bf}
    if bf3:
        for nm, arr in (("cbt", cbt), ("wq", wq_eff), ("wp", wp_eff),
                        ("wkt", wkt)):
            h, l = split_bf(arr)
            common[nm + "hl"] = np.ascontiguousarray(np.concatenate([h, l], axis=1))
    else:
        common.update({"cbt": cbt, "wq": wq_eff, "wp": wp_eff, "wkt": wkt})
    if use_bk:
        common["bk"] = bk.reshape(C, 1)
    if use_bv:
        common["bv"] = bv.reshape(C, 1)

    in_maps = []
    for core in range(NCORES):
        b, half = core // 2, core % 2
        hstc = np.zeros((CE, TL), np.float32)
        hstc[:C] = hs[b][:, half * TL:(half + 1) * TL]
        if use_qp_bias:
            hstc[C] = 1.0
        if bf3:
            h, l = split_bf(hstc)
            in_maps.append(
                {"hsthl": np.ascontiguousarray(np.concatenate([h, l], axis=1)),
                 **common})
        else:
            in_maps.append({"hst": hstc, **common})
    return in_maps, (use_qp_bias, use_bk, use_bv)


def _assemble(results):
    logits = np.empty((B, N, T), np.float32)
    idx = np.empty((B, 1, T), np.int32)
    z_q = np.empty((B, C, T), np.float32)
    for core in range(NCORES):
        b, half = core // 2, core % 2
        tsl = slice(half * TL, (half + 1) * TL)
        r = results[core]
        logits[b][:, tsl] = r["logits"].T
        idx[b, 0, tsl] = r["idx"][:, 0].astype(np.int64).astype(np.int32)
        z_q[b][:, tsl] = r["zqt"]
    return logits, idx, z_q


def _inject_ntff_hook():
    import antenv
    if "antenv.axon_hooks" in sys.modules:
        return
    m = types.ModuleType("antenv.axon_hooks")
    m._hook = None
    def _set(h):
        m._hook = h
    def _get():
        return m._hook
    m.set_axon_ntff_profile_hook = _set
    m.get_axon_ntff_profile_hook = _get
    sys.modules["antenv.axon_hooks"] = m
    antenv.axon_hooks = m
    try:
        from trn_agent_boot.trn_boot import _ntff_profile_via_ctypes
        m.set_axon_ntff_profile_hook(
            _ntff_profile_via_ctypes("/opt/axon/libaxon_pjrt.so")
        )
    except Exception:
        m.set_axon_ntff_profile_hook(None)


def run(inputs, trace=False):
    """Returns ((logits, idx, z_q), exec_time_ns_or_None)."""
    in_maps, flags = _prep_inputs(inputs)
    nc = _build(*flags)
    if trace:
        _inject_ntff_hook()
    res = run_bass_kernel_spmd(
        nc, in_maps, core_ids=list(range(NCORES)), trace=trace
    )
    return _assemble(res.results), (res.exec_time_ns if trace else None)


def kernel(**inputs):
    out, _ = run(inputs, trace=False)
    return out


# revision 22
# speedup vs baseline: 1.0658x; 1.0057x over previous
"""Trainium2 Bass kernel for nn_Attention_63866163692087 (vq_codebook).

Math (verified against the reference):
  logits[b,n,t] = a * sum_c Qg[bt,c] * K[n,c] + beta[bt]
    where Qg = (hs^T @ Wq + bq) * rep(a*(hs^T @ Wp + bp)),  a = 1/sqrt(d*H),
          K = cb @ Wk + bk  (Wk folded into the Q side: Qk = Qg @ Wk^T,
          logits = Qk @ cb^T + beta,  beta = Qg @ bk)
  idx = argmax_n logits          (softmax is monotonic)
  z_q[b,:,t] = cb[idx] @ Wv + bv (straight-through output is a pure gather)

Sharding: 8 cores, each takes 1024 rows of the flattened (b, t) axis
(core i -> b = i//2, t-half = i%2). No collectives.

Per-core layouts: everything is computed in a "transposed" orientation so
argmax runs along the DVE free axis and z_q comes out channel-major:
  hsT  [C, TL]     (native layout of hidden_states[b])
  QgT/QkT [C, TL]
  logits tiles [t(128), N] -> DMA to logits_loc [TL, N] (host transposes)
  zqT  [C, TL]     (direct concat on host)
"""

import os
import sys
import types

for _p in ("/opt/trn_rl_repo", "/root/.axon_site/_ro/trn_rl_repo"):
    if os.path.isdir(_p) and _p not in sys.path:
        sys.path.insert(0, _p)

import numpy as np
import ml_dtypes

import concourse.bass as bass
import concourse.mybir as mybir
from concourse.tile import TileContext
from concourse.vector_clock import ScopedClock
from concourse.masks import make_identity
from concourse.bass_utils import run_bass_kernel_spmd

B, C, T, N, H = 4, 512, 2048, 4096, 4
D = C // H                     # 128, head dim == one partition tile
P = 128
NCORES = 8
TL = (B * T) // NCORES         # 1024 local (b,t) rows per core
ALPHA = 1.0 / float(np.sqrt(D) * np.sqrt(H))

F32 = mybir.dt.float32
BF16 = mybir.dt.bfloat16
F32R = mybir.dt.float32r
U32 = mybir.dt.uint32
COPY = mybir.ActivationFunctionType.Copy
IDENT = mybir.ActivationFunctionType.Identity

# main-matmul dtype: "f32r" (1 cyc/row) or "f32" (4 cyc/row, exact)
MM_DT = os.environ.get("BASS_VQ_MMDT", "bf3")


def _patch_tail_drain():
    """Spread the kernel-tail drain's frontier waits over single-wait SP nops
    (walrus rejects >4 sync waits on one instruction)."""
    if getattr(TileContext, "_vq_drain_patched", False):
        return

    def _patched_dab(self, tick_clock, wait_clock):
        probe = self.nc.sync.nop()
        wait_clock.add_sem_waits(
            probe.ins, ScopedClock({None: tick_clock.global_clock})
        )
        si = probe.ins.sync_info
        if si is not None and si.on_wait is not None and len(si.on_wait) > 1:
            waits = list(si.on_wait)
            probe.ins.sync_info = mybir.SyncInfo(
                on_wait=waits[:1], on_update=list(si.on_update or [])
            )
            for w in waits[1:]:
                extra = self.nc.sync.nop()
                extra.ins.sync_info = mybir.SyncInfo(on_wait=[w], on_update=[])
        self.nc.sync.drain()
        self.nc.all_engine_barrier()
        assert self.sems is not None
        popped = self.nc._tile_sem_poison_stack.pop()
        assert popped is self._sem_poison
        self.nc.clear_and_free_semaphores(list(self.sems.allocated().values()))
        self.nc.all_engine_barrier()

    TileContext._drain_and_barrier = _patched_dab
    TileContext._vq_drain_patched = True


def _legalize_waits(nc, max_waits=1):
    """Walrus accepts only a limited number of sync waits per instruction
    (fused fp32 matmuls appear to accept just one). Move excess waits onto
    freshly inserted same-engine NOPs immediately before the instruction —
    engines execute in order, so semantics are preserved."""
    k = 0
    for f in nc.m.functions:
        for blk in f.blocks:
            il = blk.instructions
            i = 0
            while i < len(il):
                inst = il[i]
                si = getattr(inst, "sync_info", None)
                eng = getattr(inst, "engine", None)
                if (
                    si is not None
                    and si.on_wait is not None
                    and len(si.on_wait) > max_waits
                    and eng is not None
                    and eng != mybir.EngineType.Unassigned
                ):
                    waits = list(si.on_wait)
                    keep, excess = waits[:max_waits], waits[max_waits:]
                    inst.sync_info = mybir.SyncInfo(
                        on_wait=keep, on_update=list(si.on_update or [])
                    )
                    for w in excess:
                        nop = mybir.InstNoOp(
                            name=f"I-waitfix-{k}", ins=[], outs=[]
                        )
                        k += 1
                        nop.engine = eng
                        nop.sync_info = mybir.SyncInfo(on_wait=[w], on_update=[])
                        il.insert(i, nop)
                        i += 1
                i += 1
    return k


def _build(use_qp_bias, use_bk, use_bv, mm_dt=None):
    """Per-core Bass graph (SPMD across 8 cores).

    mm_dt: 'f32' — exact fp32 matmuls (4 cyc/row) on the logits chain
           'bf3' — bf16 hi/lo 3-product split (3x 1 cyc/row, near-exact)
    """
    if mm_dt is None:
        mm_dt = MM_DT
    _patch_tail_drain()
    nc = bass.Bass()
    E = 5 if use_qp_bias else 4
    CE = E * P
    bf3 = mm_dt == "bf3"
    MDT = BF16 if bf3 else F32
    # (lhs split, rhs split) product passes: x = h + l, drop l*l
    PASSES = [(0, 0), (0, 1), (1, 0)] if bf3 else [(0, 0)]
    NSP = 2 if bf3 else 1

    def dparam(name, shape, dt):
        return nc.declare_dram_parameter(name, shape, dt, isOutput=False)

    if bf3:
        # h and l halves concatenated along the free axis: one DMA per tile
        # pair (separate small loads serialize on the issue queue at startup)
        hst_d = dparam("hsthl", [CE, 2 * TL], BF16)
        cbt_d = dparam("cbthl", [C, 2 * N], BF16)
        wq_d = dparam("wqhl", [CE, 2 * C], BF16)
        wp_d = dparam("wphl", [CE, 2 * H], BF16)
        wkt_d = dparam("wkthl", [C, 2 * C], BF16)
    else:
        hst_d = [dparam("hst", [CE, TL], F32)]
        cbt_d = [dparam("cbt", [C, N], F32)]
        wq_d = [dparam("wq", [CE, C], F32)]
        wp_d = [dparam("wp", [CE, H], F32)]
        wkt_d = [dparam("wkt", [C, C], F32)]
    cb_d = dparam("cb", [N, C], F32)
    wv_d = dparam("wv", [C, C], BF16)
    if use_bk:
        bk_d = dparam("bk", [C, 1], F32)
    if use_bv:
        bv_d = dparam("bv", [C, 1], F32)

    logits_d = nc.declare_dram_parameter("logits", [TL, N], F32, isOutput=True)
    idx_d = nc.declare_dram_parameter("idx", [TL, 1], U32, isOutput=True)
    zqt_d = nc.declare_dram_parameter("zqt", [C, TL], F32, isOutput=True)

    with TileContext(nc) as tc:
        with (
            tc.tile_pool(name="const", bufs=1) as const,
            tc.tile_pool(name="work", bufs=1) as work,
            tc.tile_pool(name="gates", bufs=2) as gpool,
            tc.tile_pool(name="lrow", bufs=2) as lpool,
            tc.tile_pool(name="small", bufs=2) as spool,
            tc.tile_pool(name="zq", bufs=2) as zpool,
            tc.tile_pool(name="psmm", bufs=4, space="PSUM") as psmm,
            tc.tile_pool(name="pstp", bufs=2, space="PSUM") as pstp,
            tc.tile_pool(name="pssm", bufs=2, space="PSUM") as pssm,
        ):
            # ---------------- loads ----------------
            hst, wq, wp4, wkt, cbt = {}, {}, {}, {}, {}
            if bf3:
                hst_t = [const.tile([P, 2 * TL], BF16, name=f"hst_{e}") for e in range(E)]
                wq_t = [const.tile([P, 2 * C], BF16, name=f"wq_{e}") for e in range(E)]
                wp_t = [const.tile([P, 2 * H], BF16, name=f"wp_{e}") for e in range(E)]
                wkt_t = [const.tile([P, 2 * C], BF16, name=f"wkt_{c}") for c in range(4)]
                cbt_t = [const.tile([P, 2 * N], BF16, name=f"cbt_{e}") for e in range(4)]
                # gate matmuls run first and need only hst+wp: land those
                # before the wq tiles that stage A's Q matmuls consume later
                for e in range(E):
                    nc.sync.dma_start(wp_t[e][:], wp_d[e * P:(e + 1) * P, :])
                    nc.sync.dma_start(hst_t[e][:], hst_d[e * P:(e + 1) * P, :])
                for e in range(E):
                    nc.sync.dma_start(wq_t[e][:], wq_d[e * P:(e + 1) * P, :])
                for c4 in range(4):
                    nc.sync.dma_start(wkt_t[c4][:], wkt_d[c4 * P:(c4 + 1) * P, :])
                for c4 in range(4):
                    nc.sync.dma_start(cbt_t[c4][:], cbt_d[c4 * P:(c4 + 1) * P, :])
                for sp in range(NSP):
                    hst[sp] = [hst_t[e][:, sp * TL:(sp + 1) * TL] for e in range(E)]
                    wq[sp] = [wq_t[e][:, sp * C:(sp + 1) * C] for e in range(E)]
                    wp4[sp] = [wp_t[e][:, sp * H:(sp + 1) * H] for e in range(E)]
                    wkt[sp] = [wkt_t[c][:, sp * C:(sp + 1) * C] for c in range(4)]
                    cbt[sp] = [cbt_t[c][:, sp * N:(sp + 1) * N] for c in range(4)]
            else:
                for sp in range(NSP):
                    hst[sp] = [const.tile([P, TL], MDT, name=f"hst{sp}_{e}") for e in range(E)]
                    wq[sp] = [const.tile([P, C], MDT, name=f"wq{sp}_{e}") for e in range(E)]
                    wp4[sp] = [const.tile([P, H], MDT, name=f"wp4{sp}_{e}") for e in range(E)]
                    wkt[sp] = [const.tile([P, C], MDT, name=f"wkt{sp}_{c}") for c in range(4)]
                    cbt[sp] = [const.tile([P, N], MDT, name=f"cbt{sp}_{e}") for e in range(4)]
                    for e in range(E):
                        nc.sync.dma_start(hst[sp][e][:], hst_d[sp][e * P:(e + 1) * P, :])
                        nc.sync.dma_start(wq[sp][e][:], wq_d[sp][e * P:(e + 1) * P, :])
                        nc.sync.dma_start(wp4[sp][e][:], wp_d[sp][e * P:(e + 1) * P, :])
                    for c4 in range(4):
                        nc.sync.dma_start(wkt[sp][c4][:], wkt_d[sp][c4 * P:(c4 + 1) * P, :])
                        nc.sync.dma_start(cbt[sp][c4][:], cbt_d[sp][c4 * P:(c4 + 1) * P, :])
            wv = [const.tile([P, C], BF16, name=f"wv{c}") for c in range(4)]
            for c4 in range(4):
                nc.sync.dma_start(wv[c4][:], wv_d[c4 * P:(c4 + 1) * P, :])
            ident = const.tile([P, P], F32, name="ident")
            make_identity(nc, ident[:])
            identb = const.tile([P, P], BF16, name="identb")
            nc.vector.tensor_copy(identb[:], ident[:])
            if use_bk:
                bks = const.tile([P, 4], F32, name="bks")
                nc.sync.dma_start(bks[:], bk_d.rearrange("(c p) o -> p (c o)", p=P))
                if bf3:
                    # beta matmul operands must match qgt dtype (bf16);
                    # bk's low bf16 bits are dropped (bk==0 in practice)
                    bks_m = const.tile([P, 4], BF16, name="bksm")
                    nc.vector.tensor_copy(bks_m[:], bks[:])
                else:
                    bks_m = bks
            if use_bv:
                bvs = const.tile([P, 4], F32, name="bvs")
                nc.sync.dma_start(bvs[:], bv_d.rearrange("(c p) o -> p (c o)", p=P))

            # device-split QgT / QkT (h/l in bf16 when bf3, else plain f32)
            qgt = {sp: [work.tile([P, TL], MDT, name=f"qgt{sp}_{c}") for c in range(4)]
                   for sp in range(NSP)}
            qkt = {sp: [work.tile([P, TL], MDT, name=f"qkt{sp}_{c}") for c in range(4)]
                   for sp in range(NSP)}
            cbgt = [work.tile([P, TL], BF16, name=f"cbgt{c}") for c in range(4)]
            g4a = work.tile([P, 8 * H], F32, name="g4a")
            if use_bk:
                beta = work.tile([P, 8], F32, name="beta")

            # ------- gates: G[t, h] per t-chunk, then broadcast-transpose ----
            for i in range(8):
                g4p = pssm.tile([P, H], F32, tag="pss", name="g4p")
                nmm = len(PASSES) * E
                k = 0
                for (a, b) in PASSES:
                    for e in range(E):
                        nc.tensor.matmul(
                            g4p[:], lhsT=hst[b][e][:, i * P:(i + 1) * P],
                            rhs=wp4[a][e][:],
                            start=(k == 0), stop=(k == nmm - 1),
                        )
                        k += 1
                nc.scalar.activation(g4a[:, i * H:(i + 1) * H], g4p[:], COPY)

            # ---------------- stage A: QgT / beta / QkT ----------------
            for s in range(2):
                tsl = slice(s * 512, (s + 1) * 512)
                gsb = []
                for h in range(4):
                    grp = psmm.tile([P, 512], F32, tag="ps", name="grp")
                    for j in range(4):
                        i = s * 4 + j
                        nc.tensor.transpose(
                            grp[:, j * P:(j + 1) * P],
                            g4a[:, i * H + h:i * H + h + 1].to_broadcast([P, P]),
                            ident[:],
                        )
                    g = gpool.tile([P, 512], F32, tag="g", name="g")
                    nc.scalar.activation(g[:], grp[:], COPY)
                    gsb.append(g)
                for c4 in range(4):
                    qp = psmm.tile([P, 512], F32, tag="ps", name="qp")
                    nmm = len(PASSES) * E
                    k = 0
                    for (a, b) in PASSES:
                        for e in range(E):
                            nc.tensor.matmul(
                                qp[:], lhsT=wq[a][e][:, c4 * P:(c4 + 1) * P],
                                rhs=hst[b][e][:, tsl],
                                start=(k == 0), stop=(k == nmm - 1),
                            )
                            k += 1
                    if bf3:
                        qg32 = gpool.tile([P, 512], F32, tag="qg32", name="qg32")
                        nc.vector.tensor_mul(qg32[:], qp[:], gsb[c4][:])
                        nc.scalar.activation(qgt[0][c4][:, tsl], qg32[:], COPY)
                        nc.vector.tensor_sub(
                            qgt[1][c4][:, tsl], qg32[:], qgt[0][c4][:, tsl]
                        )
                    else:
                        nc.vector.tensor_mul(qgt[0][c4][:, tsl], qp[:], gsb[c4][:])
                if use_bk:
                    for i in range(s * 4, s * 4 + 4):
                        bp = pssm.tile([P, H], F32, tag="pss", name="bp")
                        nmm = 4 * NSP
                        k = 0
                        for sp in range(NSP):
                            for c4 in range(4):
                                nc.tensor.matmul(
                                    bp[:, 0:1],
                                    lhsT=qgt[sp][c4][:, i * P:(i + 1) * P],
                                    rhs=bks_m[:, c4:c4 + 1],
                                    start=(k == 0), stop=(k == nmm - 1),
                                )
                                k += 1
                        nc.scalar.activation(beta[:, i:i + 1], bp[:, 0:1], COPY)
                for e4 in range(4):
                    kp = psmm.tile([P, 512], F32, tag="ps", name="kp")
                    nmm = len(PASSES) * 4
                    k = 0
                    for (a, b) in PASSES:
                        for c4 in range(4):
                            nc.tensor.matmul(
                                kp[:], lhsT=wkt[a][c4][:, e4 * P:(e4 + 1) * P],
                                rhs=qgt[b][c4][:, tsl],
                                start=(k == 0), stop=(k == nmm - 1),
                            )
                            k += 1
                    nc.scalar.activation(qkt[0][e4][:, tsl], kp[:], COPY)
                    if bf3:
                        nc.vector.tensor_sub(
                            qkt[1][e4][:, tsl], kp[:], qkt[0][e4][:, tsl]
                        )

            # ---------------- stage C helper: z_q for one t-half -------------
            def stage_c(s):
                tsl_c = slice(s * 512, (s + 1) * 512)
                for c4 in range(4):
                    zp = psmm.tile([P, 512], F32, tag="ps", name="zp")
                    for e4 in range(4):
                        nc.tensor.matmul(
                            zp[:], lhsT=wv[e4][:, c4 * P:(c4 + 1) * P],
                            rhs=cbgt[e4][:, tsl_c],
                            start=(e4 == 0), stop=(e4 == 3),
                        )
                    z = zpool.tile([P, 512], F32, tag="z", name="z")
                    if use_bv:
                        nc.scalar.activation(z[:], zp[:], IDENT, bias=bvs[:, c4:c4 + 1])
                    else:
                        nc.scalar.activation(z[:], zp[:], COPY)
                    nc.sync.dma_start(zqt_d[c4 * P:(c4 + 1) * P, tsl_c], z[:])

            # ---------------- stage B: logits, argmax, gather ----------------
            cbgs = []
            for i in range(8):
                tsl = slice(i * P, (i + 1) * P)
                lsb = lpool.tile([P, N], F32, tag="l", name="l")
                for n8 in range(8):
                    nsl = slice(n8 * 512, (n8 + 1) * 512)
                    lp = psmm.tile([P, 512], F32, tag="ps", name="lp")
                    nmm = len(PASSES) * 4
                    k = 0
                    for (a, b) in PASSES:
                        for e4 in range(4):
                            nc.tensor.matmul(
                                lp[:], lhsT=qkt[a][e4][:, tsl],
                                rhs=cbt[b][e4][:, nsl],
                                start=(k == 0), stop=(k == nmm - 1),
                            )
                            k += 1
                    if use_bk:
                        nc.scalar.activation(
                            lsb[:, nsl], lp[:], IDENT, bias=beta[:, i:i + 1]
                        )
                    else:
                        nc.scalar.activation(lsb[:, nsl], lp[:], COPY)
                nc.sync.dma_start(logits_d[tsl, :], lsb[:])
                m8 = spool.tile([P, 8], F32, tag="m8", name="m8")
                i8 = spool.tile([P, 8], U32, tag="i8", name="i8")
                nc.vector.max(out=m8[:], in_=lsb[:])
                nc.vector.max_index(out=i8[:], in_max=m8[:], in_values=lsb[:])
                nc.sync.dma_start(idx_d[tsl, :], i8[:, 0:1])
                cbg = spool.tile([P, C], F32, tag="cbg", name="cbg")
                nc.gpsimd.indirect_dma_start(
                    out=cbg[:], out_offset=None, in_=cb_d[:, :],
                    in_offset=bass.IndirectOffsetOnAxis(ap=i8[:, 0:1], axis=0),
                )
                cbgb = work.tile([P, C], BF16, name=f"cbgb{i}")
                nc.vector.tensor_copy(cbgb[:], cbg[:])
                cbgs.append(cbgb)

            # ---- deferred z_q tail: keep gather->transpose->zmm off the
            # per-chunk PE critical path (PE is in-order; a transpose waiting
            # on chunk i's gather would block chunk i+1's matmuls)
            for i in range(8):
                tsl = slice(i * P, (i + 1) * P)
                for e4 in range(4):
                    tp = pstp.tile([P, P], BF16, tag="pst", name="tp")
                    nc.tensor.transpose(tp[:], cbgs[i][:, e4 * P:(e4 + 1) * P], identb[:])
                    nc.scalar.activation(cbgt[e4][:, tsl], tp[:], COPY)
                if i == 3:
                    stage_c(0)
                elif i == 7:
                    stage_c(1)



    _legalize_waits(nc)
    return nc


def _prep_inputs(inputs):
    """Host-side prep shared by all cores + per-core hs slices."""
    hs = np.ascontiguousarray(np.asarray(inputs["hidden_states"], dtype=np.float32))
    cb = np.ascontiguousarray(
        np.asarray(inputs["codebook_hidden_states"], dtype=np.float32)
    )
    Wq = np.asarray(inputs["Wq"], dtype=np.float32)
    bq = np.asarray(inputs["bq"], dtype=np.float32)
    Wk = np.asarray(inputs["Wk"], dtype=np.float32)
    bk = np.asarray(inputs["bk"], dtype=np.float32)
    Wv = np.asarray(inputs["Wv"], dtype=np.float32)
    bv = np.asarray(inputs["bv"], dtype=np.float32)
    Wp = np.asarray(inputs["Wp"], dtype=np.float32)
    bp = np.asarray(inputs["bp"], dtype=np.float32)

    use_qp_bias = bool(np.any(bq != 0) or np.any(bp != 0))
    use_bk = bool(np.any(bk != 0))
    use_bv = bool(np.any(bv != 0))

    E = 5 if use_qp_bias else 4
    CE = E * P

    wq_eff = np.zeros((CE, C), np.float32)
    wq_eff[:C] = Wq
    wp_eff = np.zeros((CE, H), np.float32)
    wp_eff[:C] = ALPHA * Wp
    if use_qp_bias:
        wq_eff[C] = bq
        wp_eff[C] = ALPHA * bp

    cbt = np.ascontiguousarray(cb.T)
    wkt = np.ascontiguousarray(Wk.T)
    wv_bf = Wv.astype(ml_dtypes.bfloat16)
    bf3 = MM_DT == "bf3"

    def split_bf(x):
        h = x.astype(ml_dtypes.bfloat16)
        l = (x - h.astype(np.float32)).astype(ml_dtypes.bfloat16)
        return h, l

    common = {"cb": cb, "wv": wv_bf}
    if bf3:
        for nm, arr in (("cbt", cbt), ("wq", wq_eff), ("wp", wp_eff),
                        ("wkt", wkt)):
            h, l = split_bf(arr)
            common[nm + "hl"] = np.ascontiguousarray(np.concatenate([h, l], axis=1))
    else:
        common.update({"cbt": cbt, "wq": wq_eff, "wp": wp_eff, "wkt": wkt})
    if use_bk:
        common["bk"] = bk.reshape(C, 1)
    if use_bv:
        common["bv"] = bv.reshape(C, 1)

    in_maps = []
    for core in range(NCORES):
        b, half = core // 2, core % 2
        hstc = np.zeros((CE, TL), np.float32)
        hstc[:C] = hs[b][:, half * TL:(half + 1) * TL]
        if use_qp_bias:
            hstc[C] = 1.0
        if bf3:
            h, l = split_bf(hstc)
            in_maps.append(
                {"hsthl": np.ascontiguousarray(np.concatenate([h, l], axis=1)),
                 **common})
        else:
            in_maps.append({"hst": hstc, **common})
    return in_maps, (use_qp_bias, use_bk, use_bv)


def _assemble(results):
    logits = np.empty((B, N, T), np.float32)
    idx = np.empty((B, 1, T), np.int32)
    z_q = np.empty((B, C, T), np.float32)
    for core in range(NCORES):
        b, half = core // 2, core % 2
        tsl = slice(half * TL, (half + 1) * TL)
        r = results[core]
        logits[b][:, tsl] = r["logits"].T
        idx[b, 0, tsl] = r["idx"][:, 0].astype(np.int64).astype(np.int32)
        z_q[b][:, tsl] = r["zqt"]
    return logits, idx, z_q


def _inject_ntff_hook():
    import antenv
    if "antenv.axon_hooks" in sys.modules:
        return
    m = types.ModuleType("antenv.axon_hooks")
    m._hook = None
    def _set(h):
        m._hook = h
    def _get():
        return m._hook
    m.set_axon_ntff_profile_hook = _set
    m.get_axon_ntff_profile_hook = _get
    sys.modules["antenv.axon_hooks"] = m
    antenv.axon_hooks = m
    try:
        from trn_agent_boot.trn_boot import _ntff_profile_via_ctypes
        m.set_axon_ntff_profile_hook(
            _ntff_profile_via_ctypes("/opt/axon/libaxon_pjrt.so")
        )
    except Exception:
        m.set_axon_ntff_profile_hook(None)


def run(inputs, trace=False):
    """Returns ((logits, idx, z_q), exec_time_ns_or_None)."""
    in_maps, flags = _prep_inputs(inputs)
    nc = _build(*flags)
    if trace:
        _inject_ntff_hook()
    res = run_bass_kernel_spmd(
        nc, in_maps, core_ids=list(range(NCORES)), trace=trace
    )
    return _assemble(res.results), (res.exec_time_ns if trace else None)


def kernel(**inputs):
    out, _ = run(inputs, trace=False)
    return out
